# revision 1
# baseline (speedup 1.0000x reference)
"""Multi-head attention (softmax over the QUERY axis) on 8 TRN2 NeuronCores.

Problem shapes: Q [T=1024, B=8, D=256]; per-head full-width projections
Wq/Wk/Wv [H=8, E=512, D=256]; Wo [D=256, H*E=4096].

Sharding: data-parallel over batch B — core b computes all H heads for
batch b. No collectives; the host re-stacks per-core outputs along B.

Per-core layout strategy (all matmul operands bf16, accum fp32 in PSUM):
  qT[e,t]  = (Wq_h @ Q_b^T) * s + bq*s   -> scores come out pre-scaled
  kT[e,t]  =  Wk_h @ Q_b^T  + bk
  AT[s,t]  =  kT^T-blocks x qT           (scores TRANSPOSED: softmax axis t
                                          = free axis -> row softmax)
  E[s,t]   =  exp(AT)        (no max-sub needed: |logits| <= ~6)
  l[s]     =  row-sum of E (fused accum_out of the Exp activation)
  V'[s,e]  = (Q_b @ Wv_h^T + bv) / l[s]
  OT[e,t]  =  V'^T-blocks x E            (= attention output, transposed)
  out[t,d] += OT^T-blocks x Wo_h^T       (accumulated over heads on DVE)
"""

import sys

sys.path.insert(0, "/opt/trn_rl_repo")

from contextlib import ExitStack

import ml_dtypes
import numpy as np

import concourse.bass as bass
import concourse.tile as tile
from concourse.tile import add_dep_helper
from concourse import bacc, bass_utils, mybir

T, B, D, H, E = 1024, 8, 256, 8, 512
N_CORES = 8

F32 = mybir.dt.float32
BF16 = mybir.dt.bfloat16
AF = mybir.ActivationFunctionType


def _bcast(ap_row, parts):
    """Partition-broadcast a [1, n] DRAM AP to [parts, n] (step-0 partition)."""
    return bass.AP(
        tensor=ap_row.tensor,
        offset=ap_row.offset,
        ap=[[0, parts], list(ap_row.ap[-1])],
    )


def build_nc(t=T, d=D, h=H, e=E):
    """Build the per-core SPMD program. Returns a compiled Bacc."""
    TB = t // 128   # t blocks (partition tiles of out / lhsT slices)
    TC = t // 512   # t chunks (512-wide matmul free dim)
    SB = t // 128   # s blocks (keys == queries length)
    EB = e // 128   # e blocks
    DC = d // 128   # d chunks (contraction for projections)

    nc = bacc.Bacc("TRN2", target_bir_lowering=False, debug=False)

    # All big inputs arrive pre-arranged partition-major ([128, free...])
    # so every load is a clean 2D DMA with one contiguous row per partition.
    qt_d = nc.dram_tensor("qt", [128, DC, t], BF16, kind="ExternalInput").ap()
    wqt_d = nc.dram_tensor("wqt", [h, 128, DC, e], BF16, kind="ExternalInput").ap()
    wkt_d = nc.dram_tensor("wkt", [h, 128, DC, e], BF16, kind="ExternalInput").ap()
    wvt_d = nc.dram_tensor("wvt", [h, 128, DC, e], BF16, kind="ExternalInput").ap()
    wot_d = nc.dram_tensor("wot", [h, 128, EB, d], BF16, kind="ExternalInput").ap()
    bq_d = nc.dram_tensor("bqs", [128, h, EB], F32, kind="ExternalInput").ap()
    bk_d = nc.dram_tensor("bks", [128, h, EB], F32, kind="ExternalInput").ap()
    bv_d = nc.dram_tensor("bv", [h, e], F32, kind="ExternalInput").ap()
    bo_d = nc.dram_tensor("bo", [d], F32, kind="ExternalInput").ap()
    out_d = nc.dram_tensor("out", [t, d], F32, kind="ExternalOutput").ap()

    with tile.TileContext(nc) as tc, ExitStack() as ctx:
        consts = ctx.enter_context(tc.tile_pool(name="consts", bufs=1))
        wpool = ctx.enter_context(tc.tile_pool(name="wpool", bufs=2))
        hpool = ctx.enter_context(tc.tile_pool(name="hpool", bufs=2))
        spool = ctx.enter_context(tc.tile_pool(name="spool", bufs=2))
        at_pool = ctx.enter_context(tc.tile_pool(name="at_pool", bufs=3, space="PSUM"))
        mm_pool = ctx.enter_context(tc.tile_pool(name="mm_pool", bufs=5, space="PSUM"))

        # ---- persistent loads -------------------------------------------
        qt_sb = consts.tile([128, DC, t], BF16)
        nc.sync.dma_start(out=qt_sb[:, 0, :], in_=qt_d[:, 0, :])
        # remaining d-chunks of Q^T are issued inside head 0, after wq --
        bq_sb = consts.tile([128, h, EB], F32)
        nc.sync.dma_start(out=bq_sb, in_=bq_d)
        bk_sb = consts.tile([128, h, EB], F32)
        nc.sync.dma_start(out=bk_sb, in_=bk_d)
        bo_bc = consts.tile([128, d], F32)
        nc.gpsimd.dma_start(out=bo_bc, in_=_bcast(bo_d[None, :], 128))
        out_acc = consts.tile([128, TB, d], F32)
        out_r = out_d.rearrange("(tb p) d -> p tb d", p=128)

        # ---- PE warm-up: dummy matmuls during the initial DMA wait so the
        # HAM clock-gate reaches 8/8 before real work lands ----------------
        scratch = consts.tile([128, 640], BF16)
        nc.vector.memset(scratch, 0.0)
        ps_w = mm_pool.tile([128, 512], F32, tag="mm")
        for _ in range(6):
            nc.tensor.matmul(
                ps_w, scratch[:, :128], scratch[:, 128:640], start=True, stop=True
            )

        for hh in range(h):
            # ---- per-head weights (double-buffered -> prefetch) ---------
            wq_sb = wpool.tile([128, DC, e], BF16)
            for dc in range(DC):
                nc.sync.dma_start(out=wq_sb[:, dc, :], in_=wqt_d[hh, :, dc, :])
            if hh == 0:
                for dc in range(1, DC):
                    nc.sync.dma_start(out=qt_sb[:, dc, :], in_=qt_d[:, dc, :])
            # Head 0's remaining loads are gated behind the first matmul so
            # their descriptors don't round-robin with the critical qt/wq
            # transfers in the DMA engines (cuts ~5us off the cold start).
            gated = []
            wk_sb = wpool.tile([128, DC, e], BF16)
            for dc in range(DC):
                nc.sync.dma_start(out=wk_sb[:, dc, :], in_=wkt_d[hh, :, dc, :])
            wv_sb = wpool.tile([128, DC, e], BF16)
            gated.append(nc.sync.dma_start(out=wv_sb, in_=wvt_d[hh]))
            wo_sb = wpool.tile([128, EB, d], BF16)
            gated.append(nc.sync.dma_start(out=wo_sb, in_=wot_d[hh]))
            bv_bc = wpool.tile([128, e], F32)
            gated.append(
                nc.gpsimd.dma_start(out=bv_bc, in_=_bcast(bv_d[hh][None, :], 128))
            )

            # ---- q/k projections, transposed [e, t] ---------------------
            qT = hpool.tile([128, EB, t], BF16)
            kT = hpool.tile([128, EB, t], BF16)
            first_mm = None
            for eb in range(EB):
                for tch in range(TC):
                    tsl = slice(tch * 512, (tch + 1) * 512)
                    ps_q = mm_pool.tile([128, 512], F32, tag="mm")
                    for dc in range(DC):
                        mm = nc.tensor.matmul(
                            ps_q,
                            wq_sb[:, dc, eb * 128 : (eb + 1) * 128],
                            qt_sb[:, dc, tsl],
                            start=(dc == 0),
                            stop=(dc == DC - 1),
                        )
                        if first_mm is None:
                            first_mm = mm
                    # bias add (per-partition) + fp32->bf16 on DVE
                    nc.vector.tensor_scalar_add(
                        qT[:, eb, tsl], ps_q, bq_sb[:, hh, eb : eb + 1]
                    )
            if hh == 0:
                for g in gated:
                    add_dep_helper(
                        g.ins, first_mm.ins, reason="defer bulk load past cold start"
                    )
            for eb in range(EB):
                for tch in range(TC):
                    tsl = slice(tch * 512, (tch + 1) * 512)
                    ps_k = mm_pool.tile([128, 512], F32, tag="mm")
                    for dc in range(DC):
                        nc.tensor.matmul(
                            ps_k,
                            wk_sb[:, dc, eb * 128 : (eb + 1) * 128],
                            qt_sb[:, dc, tsl],
                            start=(dc == 0),
                            stop=(dc == DC - 1),
                        )
                    nc.scalar.activation(
                        kT[:, eb, tsl],
                        ps_k,
                        AF.Identity,
                        bias=bk_sb[:, hh, eb : eb + 1],
                    )

            # ---- scores (transposed), exp, rowsum, V --------------------
            # ---- V projection (independent of qT/kT: fills the PE gap
            # while the last q/k PSUM->SBUF copies drain) ----------------
            Vf = hpool.tile([128, SB, e], F32)
            for sb in range(SB):
                ssl = slice(sb * 128, (sb + 1) * 128)
                ps_v = mm_pool.tile([128, 512], F32, tag="mm")
                for dc in range(DC):
                    nc.tensor.matmul(
                        ps_v,
                        qt_sb[:, dc, ssl],
                        wv_sb[:, dc, :],
                        start=(dc == 0),
                        stop=(dc == DC - 1),
                    )
                nc.vector.tensor_add(Vf[:, sb, :], ps_v, bv_bc)

            Ex = hpool.tile([128, SB, t], BF16)
            Vv = hpool.tile([128, SB, e], BF16)
            lsum2 = spool.tile([128, SB, TC], F32)
            lsum = spool.tile([128, SB], F32)
            rr = spool.tile([128, SB], F32)
            for sb in range(SB):
                ssl = slice(sb * 128, (sb + 1) * 128)
                for tch in range(TC):
                    tsl = slice(tch * 512, (tch + 1) * 512)
                    at = at_pool.tile([128, 512], F32, tag="at")
                    for eb in range(EB):
                        nc.tensor.matmul(
                            at,
                            kT[:, eb, ssl],
                            qT[:, eb, tsl],
                            start=(eb == 0),
                            stop=(eb == EB - 1),
                        )
                    nc.scalar.activation(
                        Ex[:, sb, tsl],
                        at,
                        AF.Exp,
                        accum_out=lsum2[:, sb, tch : tch + 1],
                    )
                if TC == 1:
                    nc.vector.reciprocal(rr[:, sb : sb + 1], lsum2[:, sb, 0:1])
                else:
                    nc.vector.reduce_sum(
                        lsum[:, sb : sb + 1],
                        lsum2[:, sb, :],
                        axis=mybir.AxisListType.X,
                    )
                    nc.vector.reciprocal(rr[:, sb : sb + 1], lsum[:, sb : sb + 1])
                nc.vector.tensor_scalar_mul(
                    Vv[:, sb, :], Vf[:, sb, :], rr[:, sb : sb + 1]
                )

            # ---- attention output, transposed [e, t] --------------------
            OTs = hpool.tile([128, EB, t], BF16)
            for tch in range(TC):
                tsl = slice(tch * 512, (tch + 1) * 512)
                for eb in range(EB):
                    ps_o = mm_pool.tile([128, 512], F32, tag="mm")
                    for sc in range(SB):
                        nc.tensor.matmul(
                            ps_o,
                            Vv[:, sc, eb * 128 : (eb + 1) * 128],
                            Ex[:, sc, tsl],
                            start=(sc == 0),
                            stop=(sc == SB - 1),
                        )
                    nc.scalar.activation(OTs[:, eb, tsl], ps_o, AF.Copy)

            # ---- output projection, accumulated over heads --------------
            for tb in range(TB):
                ps_p = mm_pool.tile([128, 512], F32, tag="mm")
                for eb in range(EB):
                    nc.tensor.matmul(
                        ps_p[:, :d],
                        OTs[:, eb, tb * 128 : (tb + 1) * 128],
                        wo_sb[:, eb, :],
                        start=(eb == 0),
                        stop=(eb == EB - 1),
                    )
                if hh == 0:
                    nc.vector.tensor_add(out_acc[:, tb, :], ps_p[:, :d], bo_bc)
                else:
                    nc.vector.tensor_add(out_acc[:, tb, :], out_acc[:, tb, :], ps_p[:, :d])
                if hh == h - 1:
                    # overlap output store with the remaining t-blocks
                    nc.sync.dma_start(out=out_r[:, tb, :], in_=out_acc[:, tb, :])

    nc.compile()
    return nc


_NC_CACHE = {}


def _get_nc(shape_key):
    if shape_key not in _NC_CACHE:
        _NC_CACHE[shape_key] = build_nc(*shape_key)
    return _NC_CACHE[shape_key]


def _pmajor(a, last):
    """[..., C*128, last] -> [..., 128, C, last] partition-major layout."""
    lead = a.shape[:-2]
    c = a.shape[-2] // 128
    return np.ascontiguousarray(
        a.reshape(*lead, c, 128, last).swapaxes(-3, -2)
    )


def _prep_inputs(Q, Wq, bq, Wk, bk, Wv, bv, Wo, bo):
    t, b, d = Q.shape
    h, e, _ = Wq.shape
    s = np.float32(1.0 / np.sqrt(e))
    bf = ml_dtypes.bfloat16
    Q = np.asarray(Q, np.float32)
    # [B, 128, DC, T] partition-major Q^T per batch
    qt_all = _pmajor(Q.transpose(1, 2, 0).astype(bf), t)
    wqt = _pmajor((np.asarray(Wq, np.float32).transpose(0, 2, 1) * s).astype(bf), e)
    wkt = _pmajor(np.asarray(Wk, np.float32).transpose(0, 2, 1).astype(bf), e)
    wvt = _pmajor(np.asarray(Wv, np.float32).transpose(0, 2, 1).astype(bf), e)
    wot = _pmajor(np.asarray(Wo, np.float32).T.reshape(h, e, d).astype(bf), d)
    shared = {
        "wqt": wqt,
        "wkt": wkt,
        "wvt": wvt,
        "wot": wot,
        "bqs": np.ascontiguousarray(
            (np.asarray(bq, np.float32) * s).reshape(h, -1, 128).transpose(2, 0, 1)
        ),
        "bks": np.ascontiguousarray(
            np.asarray(bk, np.float32).reshape(h, -1, 128).transpose(2, 0, 1)
        ),
        "bv": np.ascontiguousarray(np.asarray(bv, np.float32)),
        "bo": np.ascontiguousarray(np.asarray(bo, np.float32)),
    }
    in_maps = [
        {"qt": np.ascontiguousarray(qt_all[bb]), **shared} for bb in range(b)
    ]
    return in_maps, (t, d, h, e)


def kernel(Q, Wq, bq, Wk, bk, Wv, bv, Wo, bo, _trace=False):
    in_maps, (t, d, h, e) = _prep_inputs(Q, Wq, bq, Wk, bk, Wv, bv, Wo, bo)
    nc = _get_nc((t, d, h, e))
    res = bass_utils.run_bass_kernel_spmd(
        nc, in_maps, core_ids=list(range(len(in_maps))), trace=_trace
    )
    out = np.stack([res.results[b]["out"] for b in range(len(in_maps))], axis=1)
    if _trace:
        kernel.last_results = res
    return np.ascontiguousarray(out.astype(np.float32))



# revision 7
# speedup vs baseline: 1.1321x; 1.1321x over previous
"""Multi-head attention (softmax over the QUERY axis) on 8 TRN2 NeuronCores.

Problem shapes: Q [T=1024, B=8, D=256]; per-head projections Wq/Wk/Wv
[H=8, E=512, D=256]; Wo [D=256, H*E=4096]. Data-parallel over batch B.

Algebraic restructuring (exact): since o_h = attn_h @ v_h and
v_h = x@Wv_h^T + bv_h, associativity gives

    out = sum_h attn_h @ (x @ M_h^T + c_h) + bo,
    M_h = Wo_h @ Wv_h  (D x D, host-precomputed),  c_h = bv_h @ Wo_h^T.

This removes the V projection, the E-wide attn@V matmul and the output
projection (per-head MACs 1611M -> ~1142M).

fp8 (e4m3) DoubleRow is used for the two T^2 matmuls only:
  scores:  lg[s,t] = kT8^T-pairs x qT8      (q/k projected in bf16,
                                             cast to fp8 with scale aq)
  AV:      out^T[d,t] += Pn8-pairs x R8
with the low-error decomposition exp(lg) = 1 + R:
  R8 = fp8(exp(lg) - 1)  (3x less quantization error than fp8(exp)),
  Pn = (x@M^T + c) * ap/l   with l[s] = sum_t R + T  (softmax denom),
  rank-1 term  u[d] = sum_s Pn[s,d]  computed EXACTLY from fp32 P via
  tiny N=1 fp32r matmuls against rr, added once per head to a [128,DB]
  accumulator and folded into the final output pass.
"""

import sys

sys.path.insert(0, "/opt/trn_rl_repo")

from contextlib import ExitStack

import ml_dtypes
import numpy as np

import concourse.bass as bass
import concourse.tile as tile
from concourse.tile import add_dep_helper
from concourse import bacc, bass_utils, mybir

T, B, D, H, E = 1024, 8, 256, 8, 512
N_CORES = 8
AQ = 16.0       # fp8 scale on each of qT/kT (logit psum = AQ^2 * s * qk)
AP = 8192.0     # fp8 scale on Pn

F32 = mybir.dt.float32
F32R = mybir.dt.float32r
BF16 = mybir.dt.bfloat16
F8 = mybir.dt.float8e4
AF = mybir.ActivationFunctionType
ALU = mybir.AluOpType
DR = mybir.MatmulPerfMode.DoubleRow


def _bcast(ap_row, parts):
    """Partition-broadcast a [1, n] DRAM AP to [parts, n] (step-0 partition)."""
    return bass.AP(
        tensor=ap_row.tensor,
        offset=ap_row.offset,
        ap=[[0, parts], list(ap_row.ap[-1])],
    )


def build_nc(t=T, d=D, h=H, e=E):
    """Build the per-core SPMD program. Returns a compiled Bacc."""
    TC = t // 512   # t chunks (512-wide psum free dim)
    SB = t // 128   # s blocks
    EB = e // 128   # e blocks
    DC = d // 128   # d chunks (contraction for projections)
    DB = d // 128   # d blocks of the transposed output

    sc = 1.0 / (AQ * AQ)

    nc = bacc.Bacc("TRN2", target_bir_lowering=False, debug=False)

    qt_d = nc.dram_tensor("qt", [128, DC, t], BF16, kind="ExternalInput").ap()
    wqt_d = nc.dram_tensor("wqt", [h, 128, DC, e], BF16, kind="ExternalInput").ap()
    wkt_d = nc.dram_tensor("wkt", [h, 128, DC, e], BF16, kind="ExternalInput").ap()
    mt_d = nc.dram_tensor("mt", [h, 128, DC, d], BF16, kind="ExternalInput").ap()
    bq_d = nc.dram_tensor("bqs", [128, h, EB], F32, kind="ExternalInput").ap()
    bk_d = nc.dram_tensor("bks", [128, h, EB], F32, kind="ExternalInput").ap()
    cs_d = nc.dram_tensor("cs", [h, d], F32, kind="ExternalInput").ap()
    boc_d = nc.dram_tensor("boc", [128, DB], F32, kind="ExternalInput").ap()
    out_d = nc.dram_tensor("out", [d, t], F32, kind="ExternalOutput").ap()

    with tile.TileContext(nc) as tc, ExitStack() as ctx:
        consts = ctx.enter_context(tc.tile_pool(name="consts", bufs=1))
        wpool = ctx.enter_context(tc.tile_pool(name="wpool", bufs=2))
        hpool = ctx.enter_context(tc.tile_pool(name="hpool", bufs=2))
        epool = ctx.enter_context(tc.tile_pool(name="epool", bufs=4))
        spool = ctx.enter_context(tc.tile_pool(name="spool", bufs=2))
        at_pool = ctx.enter_context(tc.tile_pool(name="at_pool", bufs=3, space="PSUM"))
        mm_pool = ctx.enter_context(tc.tile_pool(name="mm_pool", bufs=4, space="PSUM"))

        # ---- persistent loads -------------------------------------------
        qt_sb = consts.tile([128, DC, t], BF16)
        nc.sync.dma_start(out=qt_sb[:, 0, :], in_=qt_d[:, 0, :])
        bq_sb = consts.tile([128, h, EB], F32)
        nc.sync.dma_start(out=bq_sb, in_=bq_d)
        bk_sb = consts.tile([128, h, EB], F32)
        nc.sync.dma_start(out=bk_sb, in_=bk_d)
        boc_sb = consts.tile([128, DB], F32)
        nc.sync.dma_start(out=boc_sb, in_=boc_d)
        out_acc = consts.tile([128, DB, t], F32)
        uacc = consts.tile([128, DB], F32)
        out_r = out_d.rearrange("(db p) t -> p db t", p=128)

        # ---- PE warm-up: dummy matmuls during the initial DMA wait ------
        scratch = consts.tile([128, 640], BF16)
        nc.vector.memset(scratch, 0.0)
        ps_w = mm_pool.tile([128, 512], F32, tag="mm")
        for _ in range(6):
            nc.tensor.matmul(
                ps_w, scratch[:, :128], scratch[:, 128:640], start=True, stop=True
            )

        for hh in range(h):
            # ---- per-head weights (double-buffered -> prefetch) ---------
            wq_sb = wpool.tile([128, DC, e], BF16)
            for dc in range(DC):
                nc.sync.dma_start(out=wq_sb[:, dc, :], in_=wqt_d[hh, :, dc, :])
            if hh == 0:
                nc.sync.dma_start(out=qt_sb[:, 1, :], in_=qt_d[:, 1, :])
            gated = []
            wk_sb = wpool.tile([128, DC, e], BF16)
            for dc in range(DC):
                nc.sync.dma_start(out=wk_sb[:, dc, :], in_=wkt_d[hh, :, dc, :])
            mt_sb = wpool.tile([128, DC, d], BF16)
            gated.append(nc.sync.dma_start(out=mt_sb, in_=mt_d[hh]))
            c_bc = wpool.tile([128, d], F32)
            gated.append(
                nc.gpsimd.dma_start(out=c_bc, in_=_bcast(cs_d[hh][None, :], 128))
            )

            # ---- q/k projections (bf16), cast to fp8 [e, t] -------------
            qT8 = hpool.tile([128, EB, t], F8)
            kT8 = hpool.tile([128, EB, t], F8)
            first_mm = None
            for eb in range(EB):
                for tch in range(TC):
                    tsl = slice(tch * 512, (tch + 1) * 512)
                    ps_q = mm_pool.tile([128, 512], F32, tag="mm")
                    for dc in range(DC):
                        mm = nc.tensor.matmul(
                            ps_q,
                            wq_sb[:, dc, eb * 128 : (eb + 1) * 128],
                            qt_sb[:, dc, tsl],
                            start=(dc == 0),
                            stop=(dc == DC - 1),
                        )
                        if first_mm is None:
                            first_mm = mm
                    nc.vector.tensor_scalar_add(
                        qT8[:, eb, tsl], ps_q, bq_sb[:, hh, eb : eb + 1]
                    )
            if hh == 0:
                for g in gated:
                    add_dep_helper(
                        g.ins, first_mm.ins, reason="defer bulk load past cold start"
                    )
            for eb in range(EB):
                for tch in range(TC):
                    tsl = slice(tch * 512, (tch + 1) * 512)
                    ps_k = mm_pool.tile([128, 512], F32, tag="mm")
                    for dc in range(DC):
                        nc.tensor.matmul(
                            ps_k,
                            wk_sb[:, dc, eb * 128 : (eb + 1) * 128],
                            qt_sb[:, dc, tsl],
                            start=(dc == 0),
                            stop=(dc == DC - 1),
                        )
                    nc.scalar.activation(
                        kT8[:, eb, tsl],
                        ps_k,
                        AF.Identity,
                        bias=bk_sb[:, hh, eb : eb + 1],
                    )

            # ---- scores (fp8 DoubleRow), R = exp - 1 --------------------
            R8 = hpool.tile([128, SB, t], F8)
            lsumR2 = spool.tile([128, SB, TC], F32)
            for sb in range(SB):
                ssl = slice(sb * 128, (sb + 1) * 128)
                for tch in range(TC):
                    tsl = slice(tch * 512, (tch + 1) * 512)
                    at = at_pool.tile([128, 512], F32, tag="at")
                    for i in range(EB // 2):
                        nc.tensor.matmul(
                            at,
                            kT8[:, 2 * i : 2 * i + 2, ssl],
                            qT8[:, 2 * i : 2 * i + 2, tsl],
                            start=(i == 0),
                            stop=(i == EB // 2 - 1),
                            perf_mode=DR,
                        )
                    et = epool.tile([128, 512], F32)
                    nc.scalar.activation(et, at, AF.Exp, scale=sc)
                    nc.vector.tensor_scalar(
                        R8[:, sb, tsl],
                        et,
                        1.0,
                        0.0,
                        op0=ALU.subtract,
                        op1=ALU.add,
                        accum_out=lsumR2[:, sb, tch : tch + 1],
                    )

            # ---- P projection (bf16): P32 = x @ M^T + c -----------------
            P32 = hpool.tile([128, SB, d], F32)
            for sb in range(SB):
                ssl = slice(sb * 128, (sb + 1) * 128)
                pp = mm_pool.tile([128, 512], F32, tag="mm")
                for dc in range(DC):
                    nc.tensor.matmul(
                        pp[:, :d],
                        qt_sb[:, dc, ssl],
                        mt_sb[:, dc, :],
                        start=(dc == 0),
                        stop=(dc == DC - 1),
                    )
                nc.vector.tensor_add(P32[:, sb, :], pp[:, :d], c_bc)

            # ---- softmax denominators: rr2 = AP / (sum_t R + T) ---------
            ls = spool.tile([128, SB], F32)
            lsS = spool.tile([128, SB], F32)
            rr2 = spool.tile([128, SB], F32)
            nc.vector.tensor_add(ls, lsumR2[:, :, 0], lsumR2[:, :, 1])
            nc.vector.tensor_scalar(
                lsS, ls, float(t), 1.0 / AP, op0=ALU.add, op1=ALU.mult
            )
            nc.vector.reciprocal(rr2, lsS)

            # ---- Pn8 = fp8(P32 * rr2) -----------------------------------
            Pn8 = hpool.tile([128, SB, d], F8)
            for sb in range(SB):
                nc.vector.tensor_scalar_mul(
                    Pn8[:, sb, :], P32[:, sb, :], rr2[:, sb : sb + 1]
                )

            # ---- AV (fp8 DoubleRow): out^T[d,t] += Pn8^T-pairs x R8 -----
            for dt in range(DB):
                dsl = slice(dt * 128, (dt + 1) * 128)
                for tch in range(TC):
                    tsl = slice(tch * 512, (tch + 1) * 512)
                    ot = at_pool.tile([128, 512], F32, tag="at")
                    for i in range(SB // 2):
                        nc.tensor.matmul(
                            ot,
                            Pn8[:, 2 * i : 2 * i + 2, dsl],
                            R8[:, 2 * i : 2 * i + 2, tsl],
                            start=(i == 0),
                            stop=(i == SB // 2 - 1),
                            perf_mode=DR,
                        )
                    if hh == 0:
                        nc.scalar.activation(out_acc[:, dt, tsl], ot, AF.Copy)
                    else:
                        nc.vector.tensor_add(
                            out_acc[:, dt, tsl], out_acc[:, dt, tsl], ot
                        )

            # ---- rank-1 term u[d] = sum_s P32[s,d]*rr2[s] (exact fp32r) -
            for dt in range(DB):
                dsl = slice(dt * 128, (dt + 1) * 128)
                up = mm_pool.tile([128, 512], F32, tag="mm")
                for sb in range(SB):
                    nc.tensor.matmul(
                        up[:, :1],
                        P32[:, sb, dsl],
                        rr2[:, sb : sb + 1],
                        start=(sb == 0),
                        stop=(sb == SB - 1),
                    )
                if hh == 0:
                    nc.scalar.activation(uacc[:, dt : dt + 1], up[:, :1], AF.Copy)
                else:
                    nc.vector.tensor_add(
                        uacc[:, dt : dt + 1], uacc[:, dt : dt + 1], up[:, :1]
                    )

        # ---- final: out = (out_acc + uacc + AP*bo) / AP, store ----------
        bvec = spool.tile([128, DB], F32)
        nc.vector.tensor_add(bvec, uacc, boc_sb)
        for dt in range(DB):
            nc.vector.tensor_scalar(
                out_acc[:, dt, :],
                out_acc[:, dt, :],
                bvec[:, dt : dt + 1],
                1.0 / AP,
                op0=ALU.add,
                op1=ALU.mult,
            )
            nc.sync.dma_start(out=out_r[:, dt, :], in_=out_acc[:, dt, :])

    nc.compile()
    return nc


_NC_CACHE = {}


def _get_nc(shape_key):
    if shape_key not in _NC_CACHE:
        _NC_CACHE[shape_key] = build_nc(*shape_key)
    return _NC_CACHE[shape_key]


def _pmajor(a, last):
    """[..., C*128, last] -> [..., 128, C, last] partition-major layout."""
    lead = a.shape[:-2]
    c = a.shape[-2] // 128
    return np.ascontiguousarray(
        a.reshape(*lead, c, 128, last).swapaxes(-3, -2)
    )


def _prep_inputs(Q, Wq, bq, Wk, bk, Wv, bv, Wo, bo):
    t, b, d = Q.shape
    h, e, _ = Wq.shape
    s = np.float32(1.0 / np.sqrt(e))
    rs_aq = np.float32(np.sqrt(s) * AQ)
    bf = ml_dtypes.bfloat16
    Q = np.asarray(Q, np.float32)
    Wq = np.asarray(Wq, np.float32)
    Wk = np.asarray(Wk, np.float32)
    Wv = np.asarray(Wv, np.float32)
    Wo = np.asarray(Wo, np.float32)
    bv = np.asarray(bv, np.float32)
    bo = np.asarray(bo, np.float32)
    # [B, 128, DC, T] partition-major x^T per batch
    qt_all = _pmajor(Q.transpose(1, 2, 0).astype(bf), t)
    wqt = _pmajor((Wq.transpose(0, 2, 1) * rs_aq).astype(bf), e)
    wkt = _pmajor((Wk.transpose(0, 2, 1) * rs_aq).astype(bf), e)
    # M_h = Wo_h @ Wv_h [D, D]; mt stores M_h^T partition-major over d'
    Wo_heads = Wo.reshape(d, h, e)
    mts = np.stack([(Wo_heads[:, hh, :] @ Wv[hh]).T for hh in range(h)])
    mt = _pmajor(mts.astype(bf), d)
    cs = np.stack([bv[hh] @ Wo_heads[:, hh, :].T for hh in range(h)])
    shared = {
        "wqt": wqt,
        "wkt": wkt,
        "mt": mt,
        "bqs": np.ascontiguousarray(
            (np.asarray(bq, np.float32) * rs_aq).reshape(h, -1, 128).transpose(2, 0, 1)
        ),
        "bks": np.ascontiguousarray(
            (np.asarray(bk, np.float32) * rs_aq).reshape(h, -1, 128).transpose(2, 0, 1)
        ),
        "cs": np.ascontiguousarray(cs.astype(np.float32)),
        "boc": np.ascontiguousarray((bo * AP).reshape(-1, 128).T.astype(np.float32)),
    }
    in_maps = [
        {"qt": np.ascontiguousarray(qt_all[bb]), **shared} for bb in range(b)
    ]
    return in_maps, (t, d, h, e)


def kernel(Q, Wq, bq, Wk, bk, Wv, bv, Wo, bo, _trace=False):
    in_maps, (t, d, h, e) = _prep_inputs(Q, Wq, bq, Wk, bk, Wv, bv, Wo, bo)
    nc = _get_nc((t, d, h, e))
    res = bass_utils.run_bass_kernel_spmd(
        nc, in_maps, core_ids=list(range(len(in_maps))), trace=_trace
    )
    # per-core output is out^T [D, T]; transpose back and stack over batch
    out = np.stack(
        [res.results[bb]["out"].T for bb in range(len(in_maps))], axis=1
    )
    if _trace:
        kernel.last_results = res
    return np.ascontiguousarray(out.astype(np.float32))
